# revision 14
# baseline (speedup 1.0000x reference)
"""FSUMGU cell on 8 Trainium2 NeuronCores — pure data-parallel variant.

Each core owns 256 batch rows and computes the ENTIRE cell for them:
no collectives at all (the AllGather of the tensor-parallel variant
costs a ~40-70us CC-stream init barrier plus 2x ~30us serialized
gathers, a large fraction of which lands on the critical path).

Layout is [batch, hidden] ("row" orientation):
    zf[b, h] = sum_k catT[k, b].T @ wfT[k, h]   (stationary actT tile,
                                                 moving 512-wide weight cols)
    fg/omf/fgx elementwise on [128b, 512h] tiles (vector engine,
                                                  broadcast bias rows)
    fgx^T for GEMM2's hidden contraction comes from a DMA XBAR
    transpose (zero tensor-engine cost).

Weights stream through a single [128, 32, 2048] bf16 arena per matrix,
time-shared k-half by k-half: phase 1 consumes wf kh0 then kh1; wn's
input-half loads into wf-kh0's slot once that half is consumed, and
wn's fgx-half into wf-kh1's slot during phase 2's input-half.

Bias rows are broadcast to 128 partitions with ones-matmuls that
double as the tensor-engine HAM warm-up.

Every matmul self-loads its stationary tile: skipping the reload via
InstMatmult.ldweights=False (without a standalone InstLdweights) races
nondeterministically on hardware, so it is disabled globally by the
post-finalize pass at the end of build().
"""
import sys

sys.path.insert(0, "/opt/trn_rl_repo")

import numpy as np
import ml_dtypes
import concourse.tile as tile
from concourse import bacc, mybir
from concourse.bass_utils import run_bass_kernel_spmd

F32 = mybir.dt.float32
BF16 = mybir.dt.bfloat16
MULT = mybir.AluOpType.mult
ADD = mybir.AluOpType.add

B, H, I = 2048, 2048, 2048
NCORES = 8
BL = B // NCORES       # 256 batch rows per core
NBT = BL // 128        # 2 batch tiles
NHB = H // 512         # 4 hidden 512-col blocks
NKH = H // 128         # 16 k-tiles, hx/fgx half
NKI = I // 128         # 16 k-tiles, input half
NK = NKH + NKI         # 32
NWARM = 30             # pure warm-up matmuls before the bias broadcasts

_NC_CACHE = None


def build():
    nc = bacc.Bacc(None, target_bir_lowering=False, debug=False)
    d_actT = nc.dram_tensor("actT", [H + I, BL], BF16, kind="ExternalInput").ap()
    d_hxr = nc.dram_tensor("hxr", [BL, H], BF16, kind="ExternalInput").ap()
    d_wfT = nc.dram_tensor("wfT", [H + I, H], BF16, kind="ExternalInput").ap()
    d_wnT = nc.dram_tensor("wnT", [H + I, H], BF16, kind="ExternalInput").ap()
    d_bias = nc.dram_tensor("bias", [128, 2 * H], BF16, kind="ExternalInput").ap()
    d_hy = nc.dram_tensor("hy", [BL, H], F32, kind="ExternalOutput").ap()

    def kmaj(dram_ap, r0, nt, c0, ncols):
        """[nt*128, ncols] DRAM slab -> [128, nt, ncols] k-major AP."""
        return dram_ap[r0:r0 + nt * 128, c0:c0 + ncols].rearrange(
            "(t p) b -> p t b", p=128)

    with tile.TileContext(nc) as tc:
        with (
            tc.tile_pool(name="const", bufs=1) as const,
            tc.tile_pool(name="warena", bufs=1) as warena,
            tc.tile_pool(name="act", bufs=1) as actp,
            tc.tile_pool(name="pers", bufs=1) as pers,
            tc.tile_pool(name="fgt", bufs=3) as fgtp,
            tc.tile_pool(name="scr", bufs=3) as scr,
            tc.tile_pool(name="ps", bufs=8, space="PSUM") as ps,
        ):
            # ---- persistent SBUF tensors
            # weight arena: two k-half slots, time-shared wf -> wn
            wA = warena.tile([128, NKH, H], BF16, tag="wA")   # wf kh0 -> wn kh1(inp)
            wB = warena.tile([128, NKH, H], BF16, tag="wB")   # wf kh1 -> wn kh0(fgx)
            s_act = actp.tile([128, NK, BL], BF16, tag="actT")     # [hx; inp]^T
            s_hxr = pers.tile([128, NBT, H], BF16, tag="hxr")
            # fgx^T lives in s_hxr's memory (tag reuse): each bt half is only
            # written after that bt's fgx elementwise product has consumed
            # the corresponding hx rows. Contiguous per-bt slab -> safe XBAR
            # transpose destination.
            s_fgxT = pers.tile([128, NBT, H], BF16, tag="hxr", name="s_fgxT")
            s_fgx = pers.tile([128, NBT, H], BF16, tag="fgx")
            s_omf = pers.tile([128, NBT, H], BF16, tag="omf")
            bias_bc = const.tile([128, 2, H], BF16, tag="biasbc")  # bfp, bn

            # ---- small loads (gpsimd): bias broadcasts, hx row-layout copy
            nc.gpsimd.dma_start(bias_bc[:], d_bias.rearrange("p (a h) -> p a h", a=2))
            nc.gpsimd.dma_start(s_hxr[:], d_hxr.rearrange("(t p) h -> p t h", p=128))

            # ---- bulk loads. sync ring: wf-kh0 x act-kh0 (kh0-critical),
            # then act-kh1, then its share of wn. scalar ring: wf-kh1 early
            # (needed only from kh1) + its share of wn + transposes + hy.
            # kt-pair interleave across BOTH HWDGE rings in strict need
            # order: [wA + act-kh0] then [wB + act-kh1]. Aggregate HBM BW
            # (~300 GB/s) is the limit; both rings must carry the critical
            # stream, earliest k-tiles first.
            for j in range(8):
                eng = nc.sync if j % 2 == 0 else nc.scalar
                kt = j * 2
                eng.dma_start(wA[:, kt:kt + 2, :], kmaj(d_wfT, kt * 128, 2, 0, H))
                eng.dma_start(s_act[:, kt:kt + 2, :],
                              kmaj(d_actT, kt * 128, 2, 0, BL))
            for j in range(8):
                eng = nc.sync if j % 2 == 0 else nc.scalar
                kt = j * 2
                eng.dma_start(s_act[:, 16 + kt:16 + kt + 2, :],
                              kmaj(d_actT, 2048 + kt * 128, 2, 0, BL))
                eng.dma_start(wB[:, kt:kt + 2, :],
                              kmaj(d_wfT, 2048 + kt * 128, 2, 0, H))

            wm = const.tile([128, 512], BF16, tag="wm")
            nc.vector.memset(wm[:], 0.0009765625)

            # ---- HAM warm-up: dependency-free matmuls sized to bridge the
            # first-DMA latency window (~14us) so real matmuls start warm
            psw = ps.tile([128, 512], F32, tag="acc", name="psw")
            for i in range(NWARM):
                nc.tensor.matmul(psw[:], wm[:, :128], wm[:],
                                 start=(i == 0), stop=(i == NWARM - 1))
            bfp_bc = bias_bc[:, 0, :]
            bn_bc = bias_bc[:, 1, :]

            # ---- phase 1: zf -> fg/omf/fgx, k-half by k-half.
            # Stationary actT tile shared across the four h-blocks.
            accs1 = {}
            for bt in range(NBT):
                for hb in range(NHB):
                    accs1[(bt, hb)] = ps.tile([128, 512], F32, tag="acc",
                                              name=f"p1acc{bt}{hb}")
            for kh, warr in ((0, wA), (1, wB)):
                for bt in range(NBT):
                    for kt in range(NKH):
                        for hb in range(NHB):
                            mm = nc.tensor.matmul(
                                accs1[(bt, hb)][:],
                                s_act[:, kh * NKH + kt, bt * 128:(bt + 1) * 128],
                                warr[:, kt, hb * 512:(hb + 1) * 512],
                                start=(kh == 0 and kt == 0),
                                stop=(kh == 1 and kt == NKH - 1))
                            if hb > 0 and not (kh == 0 and kt == 0):
                                mm.ins.ldweights = False
            # ---- wn input-half loads into wf-kh0's slot (wA now dead)
            for j in range(8):
                eng = nc.sync if j % 2 == 0 else nc.scalar
                kt = j * 2
                eng.dma_start(wA[:, kt:kt + 2, :],
                              kmaj(d_wnT, 2048 + kt * 128, 2, 0, H))
            for bt in range(NBT):
                for hb in range(NHB):
                    acc = accs1[(bt, hb)]
                    hcol = slice(hb * 512, (hb + 1) * 512)
                    fgt = fgtp.tile([128, 512], BF16, tag="fgt")
                    nc.vector.scalar_tensor_tensor(
                        fgt[:], acc[:], 0.5, bfp_bc[:, hcol], MULT, ADD)
                    nc.vector.tensor_scalar(s_omf[:, bt, hcol], fgt[:],
                                            -1.0, 1.0, MULT, ADD)
                    nc.vector.tensor_mul(s_fgx[:, bt, hcol], fgt[:],
                                         s_hxr[:, bt, hcol])
                # fgx^T via ONE whole-slab DMA XBAR transpose per b-tile
                # (contiguous destination -> safe; and only ~2us of ring
                # time each, so the wn reload DMAs behind them aren't
                # starved the way 32 per-tile transposes starved them).
                eng = nc.scalar if bt % 2 == 0 else nc.sync
                eng.dma_start_transpose(
                    s_fgxT[:, bt, :].rearrange("p (t b) -> p t b", b=128),
                    s_fgx[:, bt, :])

            # ---- wn fgx-half loads into wf-kh1's slot (wB dead after ph1)
            for j in range(8):
                eng = nc.sync if j % 2 == 0 else nc.scalar
                kt = j * 2
                eng.dma_start(wB[:, kt:kt + 2, :],
                              kmaj(d_wnT, kt * 128, 2, 0, H))

            # ---- phase 2: ng, hy. Input-half contraction first.
            accs2 = {}
            for bt in range(NBT):
                for hb in range(NHB):
                    accs2[(bt, hb)] = ps.tile([128, 512], F32, tag="acc",
                                              name=f"p2acc{bt}{hb}")
            for bt in range(NBT):
                for kt in range(NKI):
                    for hb in range(NHB):
                        mm = nc.tensor.matmul(
                            accs2[(bt, hb)][:],
                            s_act[:, NKH + kt, bt * 128:(bt + 1) * 128],
                            wA[:, kt, hb * 512:(hb + 1) * 512],
                            start=(kt == 0), stop=False)
                        if hb > 0 and kt > 0:
                            mm.ins.ldweights = False
            for bt in range(NBT):
                fT = s_fgxT[:, bt, :].rearrange("p (t b) -> p t b", b=128)
                for kt in range(NKH - 2):
                    for hb in range(NHB):
                        mm = nc.tensor.matmul(
                            accs2[(bt, hb)][:],
                            fT[:, kt, :],
                            wB[:, kt, hb * 512:(hb + 1) * 512],
                            start=False, stop=False)
                        if hb > 0:
                            mm.ins.ldweights = False
                for hb in range(NHB):
                    for kt in (NKH - 2, NKH - 1):
                        nc.tensor.matmul(
                            accs2[(bt, hb)][:],
                            fT[:, kt, :],
                            wB[:, kt, hb * 512:(hb + 1) * 512],
                            start=False, stop=(kt == NKH - 1))
                for hb in range(NHB):
                    acc = accs2[(bt, hb)]
                    hcol = slice(hb * 512, (hb + 1) * 512)
                    eng = nc.vector if hb % 2 == 0 else nc.gpsimd
                    t = scr.tile([128, 512], F32, tag="t")
                    nc.vector.tensor_add(t[:], acc[:], bn_bc[:, hcol])
                    eng.tensor_mul(t[:], t[:], s_omf[:, bt, hcol])
                    eng.tensor_add(t[:], t[:], s_fgx[:, bt, hcol])
                    nc.scalar.dma_start(
                        d_hy[bt * 128:(bt + 1) * 128, hb * 512:(hb + 1) * 512],
                        t[:])

    nc.finalize()
    # ldweights=False (skip the stationary reload when consecutive matmuls
    # share lhsT) raced nondeterministically on hardware in long streams —
    # wrong results in ~half of runs regardless of wait placement. Disable
    # it globally until the weight-slot semantics are understood.
    for blk in nc.m.functions[0].blocks:
        for inst in blk.instructions:
            if type(inst).__name__ == "InstMatmult" and inst.ldweights is False:
                inst.ldweights = None
    return nc


def _get_nc():
    global _NC_CACHE
    if _NC_CACHE is None:
        _NC_CACHE = build()
    return _NC_CACHE


def prepare_in_maps(input, hx, w_f, b_f, w_n, b_n):
    bf16 = ml_dtypes.bfloat16
    catT = np.ascontiguousarray(
        np.concatenate([hx, input], axis=1).T.astype(bf16))     # [H+I, B]
    hx16 = hx.astype(bf16)
    wfT = np.ascontiguousarray(w_f.T.astype(bf16))              # [H+I, H]
    wnT = np.ascontiguousarray(w_n.T.astype(bf16))
    bias = np.empty((128, 2 * H), dtype=bf16)
    bias[:, :H] = ((b_f + 1.0) * 0.5).astype(bf16)[None, :]
    bias[:, H:] = b_n.astype(bf16)[None, :]
    in_maps = []
    for core in range(NCORES):
        cs = slice(core * BL, (core + 1) * BL)
        in_maps.append({
            "actT": np.ascontiguousarray(catT[:, cs]),
            "hxr": np.ascontiguousarray(hx16[cs, :]),
            "wfT": wfT,
            "wnT": wnT,
            "bias": bias,
        })
    return in_maps


def assemble_output(results):
    return np.ascontiguousarray(np.concatenate(
        [np.asarray(results[c]["hy"], dtype=np.float32) for c in range(NCORES)],
        axis=0))


def kernel(input, hx, w_f, b_f, w_n, b_n, **_ignored):
    input = np.asarray(input, dtype=np.float32)
    hx = np.asarray(hx, dtype=np.float32)
    w_f = np.asarray(w_f, dtype=np.float32)
    b_f = np.asarray(b_f, dtype=np.float32)
    w_n = np.asarray(w_n, dtype=np.float32)
    b_n = np.asarray(b_n, dtype=np.float32)

    nc = _get_nc()
    in_maps = prepare_in_maps(input, hx, w_f, b_f, w_n, b_n)
    res = run_bass_kernel_spmd(nc, in_maps, list(range(NCORES)))
    return assemble_output(res.results)


if __name__ == "__main__":
    rng = np.random.default_rng(0)
    inputs = {
        "input": rng.uniform(-1, 1, (B, I)).astype(np.float32),
        "hx": rng.uniform(-1, 1, (B, H)).astype(np.float32),
        "w_f": (rng.standard_normal((H, H + I)) / np.sqrt(H + I)).astype(np.float32),
        "b_f": (rng.standard_normal(H) / np.sqrt(H + I)).astype(np.float32),
        "w_n": (rng.standard_normal((H, H + I)) / np.sqrt(H + I)).astype(np.float32),
        "b_n": (rng.standard_normal(H) / np.sqrt(H + I)).astype(np.float32),
    }
    out = kernel(**inputs)
    x64 = {k: v.astype(np.float64) for k, v in inputs.items()}
    cat = np.concatenate([x64["hx"], x64["input"]], axis=1)
    fg = (cat @ x64["w_f"].T + x64["b_f"] + 1.0) * 0.5
    fgx = fg * x64["hx"]
    ng = np.concatenate([fgx, x64["input"]], axis=1) @ x64["w_n"].T + x64["b_n"]
    exp = (1.0 - fg) * ng + fgx
    err = np.abs(out - exp).max() / np.abs(exp).max()
    print("rel err:", err)
